# revision 10
# baseline (speedup 1.0000x reference)
"""Nadaraya-Watson kernel regression (retrieval_knn) on 8 NeuronCores.

out[b,d] = sum_n y[n,d] * G(u[n,d]-v[b,d]) / sum_n G(...),
G(z) = exp(-z^2/2); u = mlp(calc_X)/h, v = mlp(x)/h (1/h folded into W2).

Sharding: N-parallel over the reference set (1024 rows/core); every core
sees all B=512 queries and returns partial num/den sums [2*DOUT, B];
the host sums partials across cores and divides (tiny [512,16] reduce).

Per-core plan (layout: n on partitions, b on free):
  - query MLP on PE -> vT [16, 512] = Zw.T/h; broadcast row d to
    VD [128, d, 512] via a DRAM bounce (v identical on all partitions).
  - reference MLP on this core's n-slice -> u_sb[p, 16c+d] = u[128c+p, d]
    ([128, 128]); usqh = -u^2/2.
  - main loop over (d, c): ONE fused ACT op makes the Gaussian weights:
      w[p, b] = Exp(VD[p,d,b] * u_sb[p,16c+d] + usqh[p,16c+d])
    (exp(uv - u^2/2); the per-(b,d) factor exp(-v^2/2) cancels in the
    num/den ratio), then ONE PE matmul with stationary [y_d|1] reduces
    over n:  psum[0,b] += sum_p w*y, psum[1,b] += sum_p w.
    fp32r moving operand -> 1 cycle/row on PE; ACT is the bottleneck.
  - after each d: DVE copies psum [2,512] into ND[2d:2d+2, :].
Host: out[b,d] = sum_c num / sum_c den, transposed to [B, DOUT].
"""
import sys
sys.path.insert(0, '/opt/trn_rl_repo')
import numpy as np
from concourse import bass, tile, bacc, mybir
from concourse.bass_utils import run_bass_kernel_spmd

F32 = mybir.dt.float32
F32R = mybir.dt.float32r
AF = mybir.ActivationFunctionType
ALU = mybir.AluOpType

B, N, DIN, DMID, DOUT = 512, 8192, 128, 256, 16
NCORES = 8
NSL = N // NCORES           # 1024 reference rows per core
NCH = NSL // 128            # 8 chunks of 128 n-rows (partition dim)


def build_kernel(reps=1, sim=False):
    nc = bacc.Bacc(None, target_bir_lowering=False)

    xT_d = nc.dram_tensor("xT", [DIN, B], F32, kind="ExternalInput")
    XTs_d = nc.dram_tensor("XTs", [DIN, NSL], F32, kind="ExternalInput")
    ST_d = nc.dram_tensor("ST", [128, NCH * DOUT * 2], F32R, kind="ExternalInput")
    W1T_d = nc.dram_tensor("W1T", [DIN, DMID], F32, kind="ExternalInput")
    W2Ta_d = nc.dram_tensor("W2Ta", [DIN, DOUT], F32, kind="ExternalInput")
    W2Tb_d = nc.dram_tensor("W2Tb", [DIN, DOUT], F32, kind="ExternalInput")
    nd_d = nc.dram_tensor("nd_out", [2 * DOUT, B], F32, kind="ExternalOutput")

    with tile.TileContext(nc) as tc:
      for _rep in range(reps):
        with (
            tc.tile_pool(name="dram", bufs=1, space="DRAM") as dram,
            tc.tile_pool(name="const", bufs=1) as cpool,
        ):
            # ---------- loads ----------
            W1T = cpool.tile([DIN, DMID], F32)
            nc.sync.dma_start(W1T[:], W1T_d[:])
            W2Ta = cpool.tile([DIN, DOUT], F32)
            nc.sync.dma_start(W2Ta[:], W2Ta_d[:])
            W2Tb = cpool.tile([DIN, DOUT], F32)
            nc.sync.dma_start(W2Tb[:], W2Tb_d[:])
            xT = cpool.tile([DIN, B], F32)
            nc.sync.dma_start(xT[:], xT_d[:])
            XTs = cpool.tile([DIN, NSL], F32)
            nc.sync.dma_start(XTs[:], XTs_d[:])
            ST = cpool.tile([128, NCH, DOUT, 2], F32R)
            nc.sync.dma_start(ST[:], ST_d[:])

            mlp_cm = tc.tile_pool(name="mlppool", bufs=1)
            mlp = mlp_cm.__enter__()
            psum_cm = tc.tile_pool(name="mlppsum", bufs=1, space="PSUM")
            mpsum = psum_cm.__enter__()

            # ---------- query MLP: vT [16, 512] ----------
            pq = mpsum.tile([128, 2 * B], F32, tag="pq")
            for j in range(2):
                nc.tensor.matmul(pq[:, B * j:B * (j + 1)],
                                 W1T[:, 128 * j:128 * j + 128], xT[:])
            HqT = mlp.tile([128, 2 * B], F32)
            nc.vector.tensor_scalar_max(HqT[:], pq[:], 0.0)
            pz = mpsum.tile([DOUT, B], F32, tag="pz")
            nc.tensor.matmul(pz[:], W2Ta[:], HqT[:, 0:B], start=True, stop=False)
            nc.tensor.matmul(pz[:], W2Tb[:], HqT[:, B:2 * B], start=False, stop=True)
            vT = mlp.tile([DOUT, B], F32)
            nc.vector.tensor_copy(vT[:], pz[:])
            v_dram = dram.tile([DOUT, B], F32)
            nc.sync.dma_start(v_dram[:], vT[:])
            # broadcast: VD[p, d, b] = v[d, b] for all p
            VD = cpool.tile([128, DOUT, B], F32)
            for d in range(DOUT):
                nc.sync.dma_start(
                    VD[:, d, :],
                    bass.AP(v_dram[:].tensor, B * d, [[0, 128], [1, B]]))

            # ---------- reference MLP on the n-slice ----------
            ph = mpsum.tile([128, 2 * NSL], F32, tag="ph")
            for j in range(2):
                for q in range(NSL // 512):
                    s = slice(NSL * j + 512 * q, NSL * j + 512 * (q + 1))
                    nc.tensor.matmul(ph[:, s], W1T[:, 128 * j:128 * j + 128],
                                     XTs[:, 512 * q:512 * (q + 1)])
            HT = mlp.tile([128, 2 * NSL], F32)
            nc.vector.tensor_scalar_max(HT[:], ph[:], 0.0)
            # L2: u_sb[p, 16c+d] = u[128c+p, d]
            pu = mpsum.tile([128, NCH * DOUT], F32, tag="pu")
            for c in range(NCH):
                for j in range(2):
                    nc.tensor.matmul(
                        pu[:, DOUT * c:DOUT * (c + 1)],
                        HT[:, NSL * j + 128 * c:NSL * j + 128 * (c + 1)],
                        (W2Ta if j == 0 else W2Tb)[:],
                        start=(j == 0), stop=(j == 1))
            u_sb = cpool.tile([128, NCH * DOUT], F32)
            nc.vector.tensor_copy(u_sb[:], pu[:])
            usqh = cpool.tile([128, NCH * DOUT], F32)
            nc.vector.scalar_tensor_tensor(usqh[:], u_sb[:], -0.5, u_sb[:],
                                           op0=ALU.mult, op1=ALU.mult)
            psum_cm.__exit__(None, None, None)
            mlp_cm.__exit__(None, None, None)

            # ---------- main pass ----------
            # ND[r, d, b]: r=0 num, r=1 den (partition offset stays 0)
            ND = cpool.tile([2, DOUT, B], F32)
            with (
                tc.tile_pool(name="mp", bufs=1) as mp,
                tc.tile_pool(name="mpsum2", bufs=2, space="PSUM") as psum2,
            ):
                for d in range(DOUT):
                    pd = psum2.tile([2, B], F32, tag="pd")
                    for c in range(NCH):
                        w = mp.tile([128, B], F32R, tag="w", bufs=3)
                        k = DOUT * c + d
                        nc.scalar.activation(w[:], VD[:, d, :], AF.Exp,
                                             scale=u_sb[:, k:k + 1],
                                             bias=usqh[:, k:k + 1])
                        nc.tensor.matmul(pd[:], ST[:, c, d, :], w[:],
                                         start=(c == 0), stop=(c == NCH - 1))
                    nc.vector.tensor_copy(ND[:, d, :], pd[:])

            # nd_out[2d+r, b] = ND[r, d, b]
            nc.sync.dma_start(
                bass.AP(nd_d[:].tensor, 0, [[B, 2], [2 * B, DOUT], [1, B]]),
                ND[:])

    nc.compile()
    return nc


_NC = None


def prep_in_maps(inputs):
    x = np.asarray(inputs["x"], dtype=np.float32)
    calc_X = np.asarray(inputs["calc_X"], dtype=np.float32)
    calc_Y = np.asarray(inputs["calc_Y"], dtype=np.float32)
    W1 = np.asarray(inputs["W1"], dtype=np.float32)
    W2 = np.asarray(inputs["W2"], dtype=np.float32)
    h = float(np.asarray(inputs["h"], dtype=np.float32).reshape(-1)[0])

    XT = np.ascontiguousarray(calc_X.T)                 # [128, 8192]
    xT = np.ascontiguousarray(x.T)                      # [128, 512]
    W1T = np.ascontiguousarray(W1.T)                    # [128, 256]
    W2Th = np.ascontiguousarray(W2.T) / h               # [256, 16], 1/h folded
    W2Ta = np.ascontiguousarray(W2Th[0:128])
    W2Tb = np.ascontiguousarray(W2Th[128:256])

    in_maps = []
    for c in range(NCORES):
        XTs = np.ascontiguousarray(XT[:, NSL * c:NSL * (c + 1)])
        ys = calc_Y[NSL * c:NSL * (c + 1)]              # [1024, 16]
        ST = np.empty((128, NCH, DOUT, 2), dtype=np.float32)
        ST[:, :, :, 0] = ys.reshape(NCH, 128, DOUT).transpose(1, 0, 2)
        ST[:, :, :, 1] = 1.0
        in_maps.append({
            "xT": xT, "XTs": XTs,
            "ST": np.ascontiguousarray(ST.reshape(128, NCH * DOUT * 2)),
            "W1T": W1T, "W2Ta": W2Ta, "W2Tb": W2Tb,
        })
    return in_maps


def combine_results(core_outs):
    """core_outs: list of [2*DOUT, B] partial num/den arrays -> [B, DOUT]."""
    nd = np.sum([np.asarray(o, dtype=np.float64) for o in core_outs], axis=0)
    num = nd[0::2, :]                                   # [DOUT, B]
    den = nd[1::2, :]
    return np.ascontiguousarray((num / den).T).astype(np.float32)


def kernel(**inputs):
    global _NC
    in_maps = prep_in_maps(inputs)
    if _NC is None:
        _NC = build_kernel()
    res = run_bass_kernel_spmd(_NC, in_maps, core_ids=list(range(NCORES)))
    return combine_results([res.results[c]["nd_out"] for c in range(NCORES)])


if __name__ == "__main__":
    rng = np.random.default_rng(0)
    ins = {
        "x": rng.standard_normal((B, DIN), dtype=np.float32),
        "calc_X": rng.standard_normal((N, DIN), dtype=np.float32),
        "calc_Y": rng.standard_normal((N, DOUT), dtype=np.float32),
        "W1": (rng.standard_normal((DMID, DIN), dtype=np.float32) * DIN ** -0.5),
        "W2": (rng.standard_normal((DOUT, DMID), dtype=np.float32) * DMID ** -0.5),
        "h": np.array([1.5], dtype=np.float32),
    }
    out = kernel(**ins)
    def mlp(v):
        return np.maximum(v @ ins["W1"].T, 0.0) @ ins["W2"].T
    Zw = mlp(ins["x"]); Xw = mlp(ins["calc_X"])
    z = (Xw[None] - Zw[:, None]) / ins["h"][0]
    w = np.exp(-0.5 * z * z)
    ref = (w * ins["calc_Y"][None]).sum(1) / w.sum(1)
    rel = np.abs(out - ref).max() / np.abs(ref).max()
    print("rel err:", rel)


# revision 27
# speedup vs baseline: 174.3759x; 174.3759x over previous
"""Nadaraya-Watson kernel regression (retrieval_knn) on 8 NeuronCores.

out[b,d] = sum_n y[n,d] * G(u[n,d]-v[b,d]) / sum_n G(...),
G(z) = exp(-z^2/2); u = mlp(calc_X)/h, v = mlp(x)/h (1/h folded into W2).

Sharding: N-parallel over the reference set (1024 rows/core); every core
sees all B=512 queries and returns partial num/den sums; the host sums
partials across cores and divides (tiny [512,16] reduce).

Per-core plan, built for MINIMAL instruction count (measured cost here is
dominated by fixed per-rep + per-instruction overheads, not engine time):
  - one packed DMA loads W1T|W2Ta|W2Tb|xT|XTs; one fused MLP over the
    1536 columns [x.T | calc_X.T-slice] (6+6 matmuls, 2 relus) gives
    vu = [v[16,512] | u[16,1024]] / h; a DRAM bounce broadcasts it to
    V[p=(16r+d), g] (fp16) and U[p, n] (fp16).
  - main pass in layout [p=(16r+d), free=(g=64, n=1024)] with stride-0
    broadcast APs, 5 giant ops (65536 elems each):
      T1 (DVE):  W = U - V            (fp16 work tile, 128KB/partition)
      T2 (ACT):  W = DerivErf(W/sqrt2) = (2/sqrt(pi)) exp(-(u-v)^2/2)
                 (constant cancels in the num/den ratio)
      R1 (DVE):  den[p, g] = sum_n W
      T3 (DVE):  W = W * Yrep  (in-place)
      R2 (DVE):  num[p, g] = sum_n W
  - ND [128, (den 64 | num 64)] fp32 -> single output DMA.
Host: sums ND over cores; out[8g+r, d] = num[16r+d, g]/den[16r+d, g].
"""
import sys
sys.path.insert(0, '/opt/trn_rl_repo')
import numpy as np
from concourse import bass, tile, bacc, mybir
from concourse.bass_utils import run_bass_kernel_spmd

F32 = mybir.dt.float32
F16 = mybir.dt.float16
AF = mybir.ActivationFunctionType
ALU = mybir.AluOpType

B, N, DIN, DMID, DOUT = 512, 8192, 128, 256, 16
NCORES = 8
NSL = N // NCORES           # 1024 reference rows per core
NG = B // 8                 # 64 query groups; b = 8g+r, p = 16r+d
CPW = DMID + 2 * DOUT + B + NSL      # packed consts width: 1824
XOFF = DMID + 2 * DOUT               # xT offset in pack: 288
ISQ2 = float(0.5 ** 0.5)


def build_kernel(reps=1, sim=False, ng=NG, nmain=5, den_eng="A", num_eng="D"):
    nc = bacc.Bacc(None, target_bir_lowering=False)

    CP_d = nc.dram_tensor("CP", [DIN, CPW], F32, kind="ExternalInput")
    YT_d = nc.dram_tensor("YTs", [DOUT, NSL], F16, kind="ExternalInput")
    nd_d = nc.dram_tensor("nd_out", [128, 2 * NG], F32, kind="ExternalOutput")

    with tile.TileContext(nc) as tc:
      for _rep in range(reps):
        with (
            tc.tile_pool(name="dram", bufs=1, space="DRAM") as dram,
            tc.tile_pool(name="sb", bufs=1) as sb,
        ):
            CP = sb.tile([DIN, CPW], F32)
            nc.sync.dma_start(CP[:], CP_d[:])
            Yrep = sb.tile([128, NSL], F16)
            nc.sync.dma_start(
                Yrep[:], bass.AP(YT_d[:].tensor, 0,
                                 [[0, 8], [NSL, DOUT], [1, NSL]]))

            # ---- fused MLP over 1536 cols [xT | XTs]: vu = [v | u] ----
            H = sb.tile([DIN, 2, B + NSL], F32)
            with tc.tile_pool(name="ps1", bufs=1, space="PSUM") as ps1:
                for j in range(2):
                    PH = ps1.tile([DIN, B + NSL], F32, tag="ph")
                    for k in range(3):
                        nc.tensor.matmul(
                            PH[:, 512 * k:512 * (k + 1)],
                            CP[:, 128 * j:128 * (j + 1)],
                            CP[:, XOFF + 512 * k:XOFF + 512 * (k + 1)])
                    nc.vector.tensor_scalar_max(H[:, j, :], PH[:], 0.0)
            VU = sb.tile([DOUT, B + NSL], F16)
            with tc.tile_pool(name="ps2", bufs=1, space="PSUM") as ps2:
                PZ = ps2.tile([DOUT, B + NSL], F32, tag="pz")
                for k in range(3):
                    for j in range(2):
                        nc.tensor.matmul(
                            PZ[:, 512 * k:512 * (k + 1)],
                            CP[:, DMID + DOUT * j:DMID + DOUT * (j + 1)],
                            H[:, j, 512 * k:512 * (k + 1)],
                            start=(j == 0), stop=(j == 1))
                nc.vector.tensor_copy(VU[:], PZ[:])
            vu_dram = dram.tile([DOUT, B + NSL], F16)
            nc.sync.dma_start(vu_dram[:], VU[:])
            # V[16r+d, g] = v[d, 8g+r];  U[16r+d, n] = u[d, n]
            # vq_dram[16r+d, g] = v[d, 8g+r], stored pre-arranged
            vq_dram = dram.tile([128, NG], F16)
            nc.sync.dma_start(
                bass.AP(vq_dram[:].tensor, 0,
                        [[NG, DOUT], [1, NG], [NG * DOUT, 8]]),
                VU[:, 0:B].rearrange("d (g r) -> d g r", g=NG))
            V = sb.tile([128, NG], F16)
            nc.sync.dma_start(V[:], vq_dram[:])
            U = sb.tile([128, NSL], F16)
            nc.sync.dma_start(
                U[:], bass.AP(vu_dram[:].tensor, B,
                              [[0, 8], [B + NSL, DOUT], [1, NSL]]))

            # ---- main pass: 5 giant ops over [128, 64, 1024] ----
            ND = sb.tile([128, 2, NG], F32)
            # inner dim padded by 8 so [g, n] can't flatten to one 65536-count
            # AP dim (16-bit ISA num_elem field caps at 65535)
            Wt = sb.tile([128, NG, NSL + 8], F16)
            W = Wt[:, :, 0:NSL]
            Ub = U[:].rearrange("p (o n) -> p o n", o=1).broadcast_to(
                [128, ng, NSL])
            Vb = V[:, 0:ng].rearrange("p (g o) -> p g o", o=1).broadcast_to(
                [128, ng, NSL])
            Yb = Yrep[:].rearrange("p (o n) -> p o n", o=1).broadcast_to(
                [128, ng, NSL])
            Wn = Wt[:, 0:ng, 0:NSL]
            if nmain >= 1:
                nc.vector.tensor_tensor(Wn, Ub, Vb, op=ALU.subtract)
            if nmain >= 2:
                nc.scalar.activation(Wn, Wn, AF.Derivative_Erf, scale=ISQ2)
            if nmain >= 3:
                # per-group accumulation passes (TensorReduce is ~8x slower
                # per element than these op classes here): den via in-place
                # ACT Copy + accum, num via in-place DVE STT(mult Y) + accum;
                # the two engines pipeline across g.
                for g in range(ng):
                    Wg = Wt[:, g, 0:NSL]
                    de = den_eng[g % len(den_eng)]
                    if de == "A":
                        nc.scalar.activation(Wg, Wg, AF.Copy,
                                             accum_out=ND[:, 0, g:g + 1])
                    elif de == "S":
                        nc.vector.scalar_tensor_tensor(
                            Wg, Wg, 1.0, Yrep[:], op0=ALU.mult,
                            op1=ALU.bypass, accum_out=ND[:, 0, g:g + 1])
                    else:
                        eng = nc.gpsimd if de == "P" else nc.vector
                        eng.tensor_scalar(Wg, Wg, 1.0, 0.0, op0=ALU.mult,
                                          op1=ALU.add,
                                          accum_out=ND[:, 0, g:g + 1])
                    ne = num_eng[g % len(num_eng)]
                    eng = nc.gpsimd if ne == "P" else nc.vector
                    eng.scalar_tensor_tensor(
                        Wg, Wg, 1.0, Yrep[:], op0=ALU.bypass, op1=ALU.mult,
                        accum_out=ND[:, 1, g:g + 1])
            if nmain < 3:
                # debug-timing variants: keep ND written so the out DMA works
                nc.vector.tensor_copy(ND[:, 0, :], V[:])
                nc.vector.tensor_copy(ND[:, 1, :], V[:])
            nc.sync.dma_start(nd_d[:], ND[:])

    nc.compile()
    return nc


_NC = None


def prep_in_maps(inputs):
    x = np.asarray(inputs["x"], dtype=np.float32)
    calc_X = np.asarray(inputs["calc_X"], dtype=np.float32)
    calc_Y = np.asarray(inputs["calc_Y"], dtype=np.float32)
    W1 = np.asarray(inputs["W1"], dtype=np.float32)
    W2 = np.asarray(inputs["W2"], dtype=np.float32)
    h = float(np.asarray(inputs["h"], dtype=np.float32).reshape(-1)[0])

    XT = np.ascontiguousarray(calc_X.T)                 # [128, 8192]
    xT = np.ascontiguousarray(x.T)                      # [128, 512]
    W1T = np.ascontiguousarray(W1.T)                    # [128, 256]
    W2Th = np.ascontiguousarray(W2.T) / h               # [256, 16], 1/h folded
    YTf = calc_Y.T.astype(np.float16)                   # [16, 8192]

    in_maps = []
    for c in range(NCORES):
        CP = np.concatenate(
            [W1T, W2Th[0:128], W2Th[128:256], xT,
             XT[:, NSL * c:NSL * (c + 1)]], axis=1)
        in_maps.append({
            "CP": np.ascontiguousarray(CP),
            "YTs": np.ascontiguousarray(YTf[:, NSL * c:NSL * (c + 1)]),
        })
    return in_maps


def combine_results(core_outs):
    """core_outs: list of [128, 2*NG] partials -> [B, DOUT] output."""
    nd = np.sum([np.asarray(o, dtype=np.float64) for o in core_outs], axis=0)
    nd = nd.reshape(8, DOUT, 2, NG)                     # [r, d, (den|num), g]
    den = nd[:, :, 0, :]
    num = nd[:, :, 1, :]
    out = num / den                                     # [r, d, g]
    return np.ascontiguousarray(
        out.transpose(2, 0, 1).reshape(B, DOUT)).astype(np.float32)


def kernel(**inputs):
    global _NC
    in_maps = prep_in_maps(inputs)
    if _NC is None:
        _NC = build_kernel()
    res = run_bass_kernel_spmd(_NC, in_maps, core_ids=list(range(NCORES)))
    return combine_results([res.results[c]["nd_out"] for c in range(NCORES)])


if __name__ == "__main__":
    rng = np.random.default_rng(0)
    ins = {
        "x": rng.standard_normal((B, DIN), dtype=np.float32),
        "calc_X": rng.standard_normal((N, DIN), dtype=np.float32),
        "calc_Y": rng.standard_normal((N, DOUT), dtype=np.float32),
        "W1": (rng.standard_normal((DMID, DIN), dtype=np.float32) * DIN ** -0.5),
        "W2": (rng.standard_normal((DOUT, DMID), dtype=np.float32) * DMID ** -0.5),
        "h": np.array([1.5], dtype=np.float32),
    }
    out = kernel(**ins)
    def mlp(v):
        return np.maximum(v @ ins["W1"].T, 0.0) @ ins["W2"].T
    Zw = mlp(ins["x"]); Xw = mlp(ins["calc_X"])
    z = (Xw[None] - Zw[:, None]) / ins["h"][0]
    w = np.exp(-0.5 * z * z)
    ref = (w * ins["calc_Y"][None]).sum(1) / w.sum(1)
    rel = np.abs(out - ref).max() / np.abs(ref).max()
    print("rel err:", rel)


# revision 29
# speedup vs baseline: 260.8584x; 1.4960x over previous
"""Nadaraya-Watson kernel regression (retrieval_knn) on 8 NeuronCores.

out[b,d] = sum_n y[n,d] * G(u[n,d]-v[b,d]) / sum_n G(...),
G(z) = exp(-z^2/2); u = mlp(calc_X)/h, v = mlp(x)/h (1/h folded into W2).

Sharding: N-parallel over the reference set (1024 rows/core); every core
sees all B=512 queries and returns partial num/den sums; the host sums
partials across cores and divides (tiny [512,16] reduce).

Per-core plan, built for MINIMAL instruction count (measured cost here is
dominated by fixed per-rep + per-instruction overheads, not engine time):
  - one packed DMA loads W1T|W2Ta|W2Tb|xT|XTs; one fused MLP over the
    1536 columns [x.T | calc_X.T-slice] (6+6 matmuls, 2 relus) gives
    vu = [v[16,512] | u[16,1024]] / h; a DRAM bounce broadcasts it to
    V[p=(16r+d), g] (fp16) and U[p, n] (fp16).
  - main pass in layout [p=(16r+d), free=(g=64, n=1024)] with stride-0
    broadcast APs, 5 giant ops (65536 elems each):
      T1 (DVE):  W = U - V            (fp16 work tile, 128KB/partition)
      T2 (ACT):  W = DerivErf(W/sqrt2) = (2/sqrt(pi)) exp(-(u-v)^2/2)
                 (constant cancels in the num/den ratio)
      R1 (DVE):  den[p, g] = sum_n W
      T3 (DVE):  W = W * Yrep  (in-place)
      R2 (DVE):  num[p, g] = sum_n W
  - ND [128, (den 64 | num 64)] fp32 -> single output DMA.
Host: sums ND over cores; out[8g+r, d] = num[16r+d, g]/den[16r+d, g].
"""
import sys
sys.path.insert(0, '/opt/trn_rl_repo')
import numpy as np
from concourse import bass, tile, bacc, mybir
from concourse.bass_utils import run_bass_kernel_spmd

F32 = mybir.dt.float32
F16 = mybir.dt.float16
AF = mybir.ActivationFunctionType
ALU = mybir.AluOpType

B, N, DIN, DMID, DOUT = 512, 8192, 128, 256, 16
NCORES = 8
NSL = N // NCORES           # 1024 reference rows per core
NG = B // 8                 # 64 query groups; b = 8g+r, p = 16r+d
CPW = DMID + 2 * DOUT + B + NSL      # packed consts width: 1824
XOFF = DMID + 2 * DOUT               # xT offset in pack: 288
ISQ2 = float(0.5 ** 0.5)


def build_kernel(reps=1, sim=False, ng=NG, nmain=5, den_eng="A", num_eng="D"):
    nc = bacc.Bacc(None, target_bir_lowering=False)

    CP_d = nc.dram_tensor("CP", [DIN, CPW], F32, kind="ExternalInput")
    YT_d = nc.dram_tensor("YTs", [DOUT, NSL], F16, kind="ExternalInput")
    nd_d = nc.dram_tensor("nd_out", [128, 2 * NG], F32, kind="ExternalOutput")

    with tile.TileContext(nc) as tc:
      for _rep in range(reps):
        with (
            tc.tile_pool(name="dram", bufs=1, space="DRAM") as dram,
            tc.tile_pool(name="sb", bufs=1) as sb,
        ):
            CP = sb.tile([DIN, CPW], F32)
            nc.sync.dma_start(CP[:], CP_d[:])
            Yrep = sb.tile([128, NSL], F16)
            nc.sync.dma_start(
                Yrep[:], bass.AP(YT_d[:].tensor, 0,
                                 [[0, 8], [NSL, DOUT], [1, NSL]]))

            # ---- fused MLP over 1536 cols [xT | XTs]: vu = [v | u] ----
            H = sb.tile([DIN, 2, B + NSL], F32)
            with tc.tile_pool(name="ps1", bufs=1, space="PSUM") as ps1:
                for j in range(2):
                    PH = ps1.tile([DIN, B + NSL], F32, tag="ph")
                    for k in range(3):
                        nc.tensor.matmul(
                            PH[:, 512 * k:512 * (k + 1)],
                            CP[:, 128 * j:128 * (j + 1)],
                            CP[:, XOFF + 512 * k:XOFF + 512 * (k + 1)])
                    nc.vector.tensor_scalar_max(H[:, j, :], PH[:], 0.0)
            VU = sb.tile([DOUT, B + NSL], F16)
            with tc.tile_pool(name="ps2", bufs=1, space="PSUM") as ps2:
                PZ = ps2.tile([DOUT, B + NSL], F32, tag="pz")
                for k in range(3):
                    for j in range(2):
                        nc.tensor.matmul(
                            PZ[:, 512 * k:512 * (k + 1)],
                            CP[:, DMID + DOUT * j:DMID + DOUT * (j + 1)],
                            H[:, j, 512 * k:512 * (k + 1)],
                            start=(j == 0), stop=(j == 1))
                nc.vector.tensor_copy(VU[:], PZ[:])
            vu_dram = dram.tile([DOUT, B + NSL], F16)
            nc.sync.dma_start(vu_dram[:], VU[:])
            # V[16r+d, g] = v[d, 8g+r];  U[16r+d, n] = u[d, n]
            # vq_dram[16r+d, g] = v[d, 8g+r], stored pre-arranged
            vq_dram = dram.tile([128, NG], F16)
            nc.sync.dma_start(
                bass.AP(vq_dram[:].tensor, 0,
                        [[NG, DOUT], [1, NG], [NG * DOUT, 8]]),
                VU[:, 0:B].rearrange("d (g r) -> d g r", g=NG))
            V = sb.tile([128, NG], F16)
            nc.sync.dma_start(V[:], vq_dram[:])
            U = sb.tile([128, NSL], F16)
            nc.sync.dma_start(
                U[:], bass.AP(vu_dram[:].tensor, B,
                              [[0, 8], [B + NSL, DOUT], [1, NSL]]))

            # ---- main pass: 5 giant ops over [128, 64, 1024] ----
            ND = sb.tile([128, 2, NG], F32)
            # inner dim padded by 8 so [g, n] can't flatten to one 65536-count
            # AP dim (16-bit ISA num_elem field caps at 65535)
            Wt = sb.tile([128, NG, NSL + 8], F16)
            W = Wt[:, :, 0:NSL]
            Ub = U[:].rearrange("p (o n) -> p o n", o=1).broadcast_to(
                [128, ng, NSL])
            Vb = V[:, 0:ng].rearrange("p (g o) -> p g o", o=1).broadcast_to(
                [128, ng, NSL])
            Yb = Yrep[:].rearrange("p (o n) -> p o n", o=1).broadcast_to(
                [128, ng, NSL])
            Wn = Wt[:, 0:ng, 0:NSL]
            if nmain >= 1:
                nc.vector.tensor_tensor(Wn, Ub, Vb, op=ALU.subtract)
            if nmain >= 2:
                nc.scalar.activation(Wn, Wn, AF.Derivative_Erf, scale=ISQ2)
            if nmain >= 3:
                # per-group accumulation passes (TensorReduce is ~8x slower
                # per element than these op classes here): den via in-place
                # ACT Copy + accum, num via in-place DVE STT(mult Y) + accum;
                # the two engines pipeline across g.
                for g in range(ng):
                    Wg = Wt[:, g, 0:NSL]
                    de = den_eng[g % len(den_eng)]
                    if de == "A":
                        nc.scalar.activation(Wg, Wg, AF.Copy,
                                             accum_out=ND[:, 0, g:g + 1])
                    elif de == "S":
                        nc.vector.scalar_tensor_tensor(
                            Wg, Wg, 1.0, Yrep[:], op0=ALU.mult,
                            op1=ALU.bypass, accum_out=ND[:, 0, g:g + 1])
                    else:
                        eng = nc.gpsimd if de == "P" else nc.vector
                        eng.tensor_scalar(Wg, Wg, 1.0, 0.0, op0=ALU.mult,
                                          op1=ALU.add,
                                          accum_out=ND[:, 0, g:g + 1])
                    ne = num_eng[g % len(num_eng)]
                    eng = nc.gpsimd if ne == "P" else nc.vector
                    eng.scalar_tensor_tensor(
                        Wg, Wg, 1.0, Yrep[:], op0=ALU.bypass, op1=ALU.mult,
                        accum_out=ND[:, 1, g:g + 1])
            if nmain < 3:
                # debug-timing variants: keep ND written so the out DMA works
                nc.vector.tensor_copy(ND[:, 0, :], V[:])
                nc.vector.tensor_copy(ND[:, 1, :], V[:])
            nc.sync.dma_start(nd_d[:], ND[:])

    nc.compile()
    return nc


_NC = None


def prep_in_maps(inputs):
    x = np.asarray(inputs["x"], dtype=np.float32)
    calc_X = np.asarray(inputs["calc_X"], dtype=np.float32)
    calc_Y = np.asarray(inputs["calc_Y"], dtype=np.float32)
    W1 = np.asarray(inputs["W1"], dtype=np.float32)
    W2 = np.asarray(inputs["W2"], dtype=np.float32)
    h = float(np.asarray(inputs["h"], dtype=np.float32).reshape(-1)[0])

    XT = np.ascontiguousarray(calc_X.T)                 # [128, 8192]
    xT = np.ascontiguousarray(x.T)                      # [128, 512]
    W1T = np.ascontiguousarray(W1.T)                    # [128, 256]
    W2Th = np.ascontiguousarray(W2.T) / h               # [256, 16], 1/h folded
    YTf = calc_Y.T.astype(np.float16)                   # [16, 8192]

    in_maps = []
    for c in range(NCORES):
        CP = np.concatenate(
            [W1T, W2Th[0:128], W2Th[128:256], xT,
             XT[:, NSL * c:NSL * (c + 1)]], axis=1)
        in_maps.append({
            "CP": np.ascontiguousarray(CP),
            "YTs": np.ascontiguousarray(YTf[:, NSL * c:NSL * (c + 1)]),
        })
    return in_maps


def combine_results(core_outs):
    """core_outs: list of [128, 2*NG] partials -> [B, DOUT] output."""
    nd = np.sum([np.asarray(o, dtype=np.float64) for o in core_outs], axis=0)
    nd = nd.reshape(8, DOUT, 2, NG)                     # [r, d, (den|num), g]
    den = nd[:, :, 0, :]
    num = nd[:, :, 1, :]
    out = num / den                                     # [r, d, g]
    return np.ascontiguousarray(
        out.transpose(2, 0, 1).reshape(B, DOUT)).astype(np.float32)


def kernel(**inputs):
    global _NC
    in_maps = prep_in_maps(inputs)
    if _NC is None:
        _NC = build_kernel()
    res = run_bass_kernel_spmd(_NC, in_maps, core_ids=list(range(NCORES)))
    return combine_results([res.results[c]["nd_out"] for c in range(NCORES)])


if __name__ == "__main__":
    rng = np.random.default_rng(0)
    ins = {
        "x": rng.standard_normal((B, DIN), dtype=np.float32),
        "calc_X": rng.standard_normal((N, DIN), dtype=np.float32),
        "calc_Y": rng.standard_normal((N, DOUT), dtype=np.float32),
        "W1": (rng.standard_normal((DMID, DIN), dtype=np.float32) * DIN ** -0.5),
        "W2": (rng.standard_normal((DOUT, DMID), dtype=np.float32) * DMID ** -0.5),
        "h": np.array([1.5], dtype=np.float32),
    }
    out = kernel(**ins)
    def mlp(v):
        return np.maximum(v @ ins["W1"].T, 0.0) @ ins["W2"].T
    Zw = mlp(ins["x"]); Xw = mlp(ins["calc_X"])
    z = (Xw[None] - Zw[:, None]) / ins["h"][0]
    w = np.exp(-0.5 * z * z)
    ref = (w * ins["calc_Y"][None]).sum(1) / w.sum(1)
    rel = np.abs(out - ref).max() / np.abs(ref).max()
    print("rel err:", rel)
